# revision 3
# baseline (speedup 1.0000x reference)
"""Mixtral sparse MoE block (B=2, S=2048, D=1024, F=4096, E=8, top-2) on
8 Trainium2 NeuronCores.

Strategy: expert-parallel, dense-per-expert. Core e holds expert e's
weights (host-sharded, pre-transposed to the matmul-native layout and
cast to bf16). Every core:
  - PE-transposes the fp32 activations (x -> xT) and computes the router
    logits in fp32 (top-k selection must match the fp32 reference),
  - derives its own expert's combined routing weight per token via a
    top-2 + sigmoid reduction (softmax-renorm over 2 == sigmoid of the
    logit difference),
  - runs the expert FFN for ALL tokens in bf16 (silu(x@w1T) * (x@w3T)
    then @w2T), scales by the routing weight (0 for unrouted tokens),
  - ReduceScatters the weighted partial outputs over the 8 cores.
The host reassembles the scattered shards into the full output.
"""
import os
import sys
import types

sys.path.insert(0, "/opt/trn_rl_repo")

import numpy as np
import ml_dtypes

import concourse.bass as bass
import concourse.mybir as mybir
import concourse.tile as tile
from concourse import bass_utils
from concourse.masks import make_identity

# ---------------------------------------------------------------------------
# Container compatibility: this walrus build accepts at most one sync-wait
# and one sync-update per instruction and rejects the eq-wait drain
# butterfly Tile emits at kernel tail. Patch the tail barrier and add a
# post-pass splitting oversized wait lists onto NoOps.
# ---------------------------------------------------------------------------
MAX_WAITS = 1
MAX_UPDATES = 1


def _install_ntff_hook():
    import antenv

    if getattr(antenv, "axon_hooks", None) is not None:
        return
    hooks = types.ModuleType("antenv.axon_hooks")
    holder = [None]
    hooks.set_axon_ntff_profile_hook = lambda h: holder.__setitem__(0, h)
    hooks.get_axon_ntff_profile_hook = lambda: holder[0]
    sys.modules["antenv.axon_hooks"] = hooks
    antenv.axon_hooks = hooks
    try:
        from trn_agent_boot.trn_boot import _ntff_profile_via_ctypes

        hooks.set_axon_ntff_profile_hook(
            _ntff_profile_via_ctypes("/opt/axon/libaxon_pjrt.so")
        )
    except Exception as e:
        print(f"kernel: NTFF hook unavailable: {e}", file=sys.stderr)


def _patched_drain_and_barrier(self, tick_clock, wait_clock):
    nc = self.nc
    drain_inst = nc.sync.drain()
    wait_clock.add_sem_waits(
        drain_inst.ins, tile.ScopedClock({None: tick_clock.global_clock})
    )
    si = drain_inst.ins.sync_info
    waits = list(si.on_wait or []) if si is not None else []
    if len(waits) > MAX_WAITS:
        drain_inst.ins.sync_info = mybir.SyncInfo(
            on_wait=waits[:MAX_WAITS], on_update=list(si.on_update or [])
        )
        rest = waits[MAX_WAITS:]
        while rest:
            extra = nc.sync.drain()
            extra.ins.sync_info = mybir.SyncInfo(on_wait=rest[:MAX_WAITS], on_update=[])
            rest = rest[MAX_WAITS:]
    nc._nrt_pseudo_barrier()
    assert self.sems is not None
    popped = nc._tile_sem_poison_stack.pop()
    assert popped is self._sem_poison
    nc.clear_and_free_semaphores(list(self.sems.allocated().values()))
    nc._nrt_pseudo_barrier()


tile.TileContext._drain_and_barrier = _patched_drain_and_barrier

_nop_counter = [0]


def _fix_sync_waits(nc):
    n_fixed = 0
    for func in nc.m.functions:
        for bb in func.blocks:
            insts = list(bb.instructions)
            out = []
            changed = False
            for ins in insts:
                si = ins.sync_info
                waits = list(si.on_wait or []) if si is not None else []
                upds = list(si.on_update or []) if si is not None else []
                pre = []
                post = []
                if len(waits) > MAX_WAITS:
                    rest, waits = waits[:-MAX_WAITS], waits[-MAX_WAITS:]
                    while rest:
                        _nop_counter[0] += 1
                        nop = mybir.InstNoOp(
                            name=f"waitsplit-{_nop_counter[0]}", ins=[], outs=[]
                        )
                        nop.engine = ins.engine
                        nop.sync_info = mybir.SyncInfo(
                            on_wait=rest[:MAX_WAITS], on_update=[]
                        )
                        rest = rest[MAX_WAITS:]
                        pre.append(nop)
                if len(upds) > MAX_UPDATES:
                    is_dma = "DMA" in type(ins).__name__ or "Dma" in type(ins).__name__
                    assert not is_dma, (
                        f"DMA instruction {ins.name} has {len(upds)} updates; "
                        "cannot split safely"
                    )
                    rest_u, upds = upds[MAX_UPDATES:], upds[:MAX_UPDATES]
                    while rest_u:
                        _nop_counter[0] += 1
                        nop = mybir.InstNoOp(
                            name=f"updsplit-{_nop_counter[0]}", ins=[], outs=[]
                        )
                        nop.engine = ins.engine
                        nop.sync_info = mybir.SyncInfo(
                            on_wait=[], on_update=rest_u[:MAX_UPDATES]
                        )
                        rest_u = rest_u[MAX_UPDATES:]
                        post.append(nop)
                if pre or post:
                    ins.sync_info = mybir.SyncInfo(on_wait=waits, on_update=upds)
                    changed = True
                    n_fixed += 1
                out.extend(pre)
                out.append(ins)
                out.extend(post)
            if changed:
                bb.instructions = out
    return n_fixed


# ---------------------------------------------------------------------------
# Problem constants (hardcoded per the grading contract).
# ---------------------------------------------------------------------------
B, S, D, F, E = 2, 2048, 1024, 4096, 8
T = B * S            # 4096 tokens
NCORES = 8
TB = 1024            # tokens per block
NB = T // TB         # 4 blocks
PC = 128             # partition chunk
DCN = D // PC        # 8 d-chunks
FCN = F // PC        # 32 f-chunks
FGW = 512            # f-group width for mm1 weight slabs
NFG = F // FGW       # 8 f-groups
NT = 512             # matmul moving free dim
F32 = mybir.dt.float32
BF16 = mybir.dt.bfloat16
AX = mybir.AxisListType.X
ALU = mybir.AluOpType
ACTF = mybir.ActivationFunctionType


def _build():
    nc = bass.Bass(num_devices=NCORES)
    x = nc.dram_tensor("x", [T, D], F32, kind="ExternalInput")
    gwt = nc.dram_tensor("gwt", [D, E], F32, kind="ExternalInput")
    eoh = nc.dram_tensor("eoh", [PC, E], F32, kind="ExternalInput")
    w1t = nc.dram_tensor("w1t", [D, F], BF16, kind="ExternalInput")
    w3t = nc.dram_tensor("w3t", [D, F], BF16, kind="ExternalInput")
    w2t = nc.dram_tensor("w2t", [F, D], BF16, kind="ExternalInput")
    out = nc.dram_tensor("out", [T // NCORES, D], F32, kind="ExternalOutput")

    with tile.TileContext(nc) as tc:
        with (
            tc.tile_pool(name="const", bufs=1) as cpool,
            tc.tile_pool(name="xtb", bufs=1) as xpool,
            tc.tile_pool(name="ht", bufs=1) as hpool,
            tc.tile_pool(name="small", bufs=3) as npool,
            tc.tile_pool(name="wslab", bufs=2) as wpool,
            tc.tile_pool(name="w2s", bufs=4) as w2pool,
            tc.tile_pool(name="stage", bufs=3) as spool,
            tc.tile_pool(name="psum", bufs=2, space="PSUM") as psum,
            tc.tile_pool(name="ypsum", bufs=1, space="PSUM") as ypsum,
            tc.tile_pool(name="dram", bufs=2, space="DRAM") as dram,
        ):
            # ---- constants ----
            ident = cpool.tile([PC, PC], F32, tag="ident")
            make_identity(nc, ident)
            gw_sb = []
            for dc in range(DCN):
                g = cpool.tile([PC, E], F32, tag=f"gw{dc}")
                nc.sync.dma_start(g, gwt[dc * PC:(dc + 1) * PC, :])
                gw_sb.append(g)
            eoh_sb = cpool.tile([PC, E], F32, tag="eoh")
            nc.sync.dma_start(eoh_sb, eoh[:, :])
            wgt = cpool.tile([PC, T // PC], F32, tag="wgt")
            xtb = [xpool.tile([PC, T], BF16, tag=f"xtb{dc}", name=f"xtb{dc}") for dc in range(DCN)]

            # ---- phase 0: transpose x (fp32), router logits, top-2 weights ----
            for tcn in range(T // PC):
                xn = npool.tile([PC, D], F32, tag="xn")
                nc.sync.dma_start(xn, x[tcn * PC:(tcn + 1) * PC, :])
                lg = ypsum.tile([PC, E], F32, tag="yp0", name="lg")
                for dc in range(DCN):
                    ptr = psum.tile([PC, PC], F32, tag=("pa" if dc % 2 == 0 else "pb"), name="ptr")
                    nc.tensor.transpose(ptr, xn[:, dc * PC:(dc + 1) * PC], ident)
                    xtf = npool.tile([PC, PC], F32, tag="xtf")
                    nc.vector.tensor_copy(xtf, ptr)
                    nc.vector.tensor_copy(xtb[dc][:, tcn * PC:(tcn + 1) * PC], ptr)
                    nc.tensor.matmul(
                        lg, xtf, gw_sb[dc], start=(dc == 0), stop=(dc == DCN - 1)
                    )
                # top-2 of the 8 logits; own-expert combined weight
                m1 = npool.tile([PC, 1], F32, tag="m1")
                nc.vector.reduce_max(m1, lg, axis=AX)
                eq1 = npool.tile([PC, E], F32, tag="eq1")
                nc.vector.tensor_scalar(eq1, lg, m1, None, op0=ALU.is_ge)
                big = npool.tile([PC, E], F32, tag="big")
                nc.vector.tensor_scalar(big, eq1, 1e30, None, op0=ALU.mult)
                lm = npool.tile([PC, E], F32, tag="lm")
                nc.vector.tensor_sub(lm, lg, big)
                m2 = npool.tile([PC, 1], F32, tag="m2")
                nc.vector.reduce_max(m2, lm, axis=AX)
                lesel = npool.tile([PC, E], F32, tag="lesel")
                nc.vector.tensor_mul(lesel, lg, eoh_sb)
                le = npool.tile([PC, 1], F32, tag="le")
                nc.vector.reduce_sum(le, lesel, axis=AX)
                is1 = npool.tile([PC, 1], F32, tag="is1")
                nc.vector.tensor_tensor(is1, le, m1, op=ALU.is_ge)
                sel = npool.tile([PC, 1], F32, tag="sel")
                nc.vector.tensor_tensor(sel, le, m2, op=ALU.is_ge)
                dmm = npool.tile([PC, 1], F32, tag="dmm")
                nc.vector.tensor_sub(dmm, m2, m1)
                oth = npool.tile([PC, 1], F32, tag="oth")
                nc.vector.tensor_mul(oth, is1, dmm)
                nc.vector.tensor_add(oth, oth, m1)
                z = npool.tile([PC, 1], F32, tag="z")
                nc.vector.tensor_sub(z, le, oth)
                sg = npool.tile([PC, 1], F32, tag="sg")
                nc.scalar.activation(sg, z, ACTF.Sigmoid)
                nc.vector.tensor_mul(wgt[:, tcn:tcn + 1], sg, sel)

            # ---- main loop over token blocks ----
            for b in range(NB):
                t0 = b * TB
                ht = [hpool.tile([PC, TB], BF16, tag=f"ht{fc}", name=f"ht{fc}") for fc in range(FCN)]
                # mm1 + mm3 -> ht (f on partitions, t on free)
                for fg in range(NFG):
                    w1s, w3s = [], []
                    for dc in range(DCN):
                        a = wpool.tile([PC, FGW], BF16, tag=f"w1s{dc}")
                        nc.sync.dma_start(
                            a, w1t[dc * PC:(dc + 1) * PC, fg * FGW:(fg + 1) * FGW]
                        )
                        w1s.append(a)
                        c = wpool.tile([PC, FGW], BF16, tag=f"w3s{dc}")
                        nc.sync.dma_start(
                            c, w3t[dc * PC:(dc + 1) * PC, fg * FGW:(fg + 1) * FGW]
                        )
                        w3s.append(c)
                    for fcl in range(FGW // PC):
                        fc = fg * (FGW // PC) + fcl
                        fsl = slice(fcl * PC, (fcl + 1) * PC)
                        for th in range(TB // NT):
                            tsl = slice(t0 + th * NT, t0 + (th + 1) * NT)
                            pa = psum.tile([PC, NT], F32, tag="pa")
                            pb = psum.tile([PC, NT], F32, tag="pb")
                            for dc in range(DCN):
                                nc.tensor.matmul(
                                    pa, w1s[dc][:, fsl], xtb[dc][:, tsl],
                                    start=(dc == 0), stop=(dc == DCN - 1),
                                )
                            for dc in range(DCN):
                                nc.tensor.matmul(
                                    pb, w3s[dc][:, fsl], xtb[dc][:, tsl],
                                    start=(dc == 0), stop=(dc == DCN - 1),
                                )
                            g = spool.tile([PC, NT], BF16, tag="g")
                            nc.scalar.activation(g, pa, ACTF.Silu)
                            nc.vector.tensor_tensor(
                                ht[fc][:, th * NT:(th + 1) * NT], g, pb, op=ALU.mult
                            )
                # mm2: y[t, dd] = sum_f ht^T w2t, scaled by routing weight
                ybuf = dram.tile([TB, D], F32, tag="ybuf")
                for dd in range(D // NT):
                    dsl = slice(dd * NT, (dd + 1) * NT)
                    for tsg in range(2):
                        yps = [
                            ypsum.tile([PC, NT], F32, tag=f"yp{tsq}", name=f"yp{tsq}")
                            for tsq in range(4)
                        ]
                        for fc in range(FCN):
                            w2s = w2pool.tile([PC, NT], BF16, tag="w2s")
                            nc.sync.dma_start(
                                w2s, w2t[fc * PC:(fc + 1) * PC, dsl]
                            )
                            for tsq in range(4):
                                ts = tsg * 4 + tsq
                                nc.tensor.matmul(
                                    yps[tsq],
                                    ht[fc][:, ts * PC:(ts + 1) * PC],
                                    w2s,
                                    start=(fc == 0),
                                    stop=(fc == FCN - 1),
                                )
                        for tsq in range(4):
                            ts = tsg * 4 + tsq
                            yst = spool.tile([PC, NT], F32, tag="yst")
                            gidx = b * (TB // PC) + ts
                            nc.vector.tensor_scalar_mul(
                                yst, yps[tsq], wgt[:, gidx:gidx + 1]
                            )
                            nc.sync.dma_start(
                                ybuf[ts * PC:(ts + 1) * PC, dsl], yst
                            )
                # sum across cores; each core keeps its 128-token shard
                yshard = dram.tile([TB // NCORES, D], F32, tag="yshard")
                nc.gpsimd.collective_compute(
                    "ReduceScatter",
                    ALU.add,
                    replica_groups=[list(range(NCORES))],
                    ins=[ybuf.opt()],
                    outs=[yshard.opt()],
                )
                nc.sync.dma_start(
                    out[b * (TB // NCORES):(b + 1) * (TB // NCORES), :], yshard
                )

    _fix_sync_waits(nc)
    return nc


_CACHED = {}


def kernel(hidden_states, gate_w, w1, w3, w2):
    _install_ntff_hook()
    if "nc" not in _CACHED:
        _CACHED["nc"] = _build()
    nc = _CACHED["nc"]

    x = np.ascontiguousarray(hidden_states.reshape(T, D)).astype(np.float32)
    gwt = np.ascontiguousarray(np.asarray(gate_w, np.float32).T)  # [D, E]
    bf = ml_dtypes.bfloat16
    in_maps = []
    for e in range(NCORES):
        eoh = np.zeros((PC, E), np.float32)
        eoh[:, e] = 1.0
        in_maps.append(
            {
                "x": x,
                "gwt": gwt,
                "eoh": eoh,
                "w1t": np.ascontiguousarray(np.asarray(w1[e]).T).astype(bf),
                "w3t": np.ascontiguousarray(np.asarray(w3[e]).T).astype(bf),
                "w2t": np.ascontiguousarray(np.asarray(w2[e]).T).astype(bf),
            }
        )

    trace = bool(int(os.environ.get("KERNEL_TRACE", "0")))
    res = bass_utils.run_bass_kernel_spmd(
        nc, in_maps, core_ids=list(range(NCORES)), trace=trace
    )
    _CACHED["last_result"] = res

    full = np.empty((T, D), np.float32)
    for r in range(NCORES):
        shard = np.asarray(res.results[r]["out"])  # [T//NCORES, D]
        for b in range(NB):
            n = TB // NCORES
            full[b * TB + r * n: b * TB + (r + 1) * n] = shard[b * n:(b + 1) * n]
    return full.reshape(B, S, D)
